# revision 23
# baseline (speedup 1.0000x reference)
"""Trainium2 Bass kernel for nn_BoundarySeg (gnn_message_passing).

Computation (per example b, position j, MAX_SPAN_LEN=6 window):
    first[j]  = sum_{d=0..5, j+d<L} w[j, j+d] * h[j+d]
    second[j] = h[j] * sum_{d, j+d<L} w[j, j+d]
    out[j]    = concat([first, second])            # [B, L, 2H]

Only the 6-diagonal band of the [B, L, L] adjacency is ever used, so the
host extracts that band (a pure strided gather / data-layout step) and
builds small banded weight matrices; all arithmetic (the windowed weighted
sums and the scaled copy) runs on-device.

Device strategy (pure data parallel, B=16 sharded 2-per-core over 8 cores):
  - 128-aligned h blocks; tile t computes out rows [128t, 128t+123) as one
    banded matmul (lhsT[k, m] = band[128t+m, k-m], zero off the diagonals)
    against h block t. The remaining 5 boundary rows per block (whose
    windows straddle the block edge) are computed by ONE batched
    block-diagonal matmul per example over the 10 consecutive h rows at
    each boundary (K=80, M=40), then placed into the staged output tiles
    with small SBUF->SBUF DMAs (engine partition-slices must be 32-aligned
    on TRN2; DMA has no such restriction).
  - `second` as a per-partition tensor_scalar multiply on the Vector
    engine, with the window sums reduced on-device from the band.
  - DMA efficiency: h and out use partition-major packed DRAM layouts
    (host packs/unpacks) giving 24KB / 12KB contiguous descriptor runs,
    and every DMA's partition count is a multiple of 16 so the descriptor
    balancer spreads each transfer across all 16 SDMA engines.
  - HBM traffic per core ~20 MB (in ~7.5 MB + out 12.6 MB): memory-bound.
"""

import os
import sys

import numpy as np

if "/opt/trn_rl_repo" not in sys.path:
    sys.path.insert(0, "/opt/trn_rl_repo")

B, L, H = 16, 1024, 768
D = 6             # MAX_SPAN_LEN
NCORES = 8
BP = B // NCORES  # examples per core
P = 128
NT = L // P       # 8 aligned tiles per example
MT = P - (D - 1)  # 123 main-matmul output rows per tile
SW = D - 1        # 5 boundary rows per block
SR = 2 * SW       # 10 h rows feeding each boundary group
SK = NT * SR      # 80 rows in the batched seam matmul (K)
SM = NT * SW      # 40 seam output rows (M)

# wband column layout
WA_COLS = NT * MT           # 984
BAND_OFF = WA_COLS          # 8*6 = 48 band cols
SEAM_OFF = BAND_OFF + NT * D
WB_F = SEAM_OFF + SM        # 1072

OUT_PAIR = 2                # output tiles per store DMA (12KB descriptors)

_nc_cache = None


def _build_bass():
    import concourse.tile as tile
    from concourse import bacc, mybir

    f32 = mybir.dt.float32
    nc = bacc.Bacc("TRN2", target_bir_lowering=False)

    h_d = nc.dram_tensor("hpack", [BP, P, NT * H], f32, kind="ExternalInput")
    hs_d = nc.dram_tensor("hseam", [BP, SK, H], f32, kind="ExternalInput")
    wband_d = nc.dram_tensor("wband", [BP, P, WB_F], f32, kind="ExternalInput")
    out_d = nc.dram_tensor("outpack", [BP, P, NT * 2 * H], f32, kind="ExternalOutput")

    with tile.TileContext(nc) as tc:
        with (
            tc.tile_pool(name="hpool", bufs=2) as hpool,
            tc.tile_pool(name="wpool", bufs=2) as wpool,
            tc.tile_pool(name="srhs", bufs=2) as srhs_pool,
            tc.tile_pool(name="ssb", bufs=2) as ssb_pool,
            tc.tile_pool(name="opool", bufs=2) as opool,
            tc.tile_pool(name="spool", bufs=4) as spool,
            tc.tile_pool(name="pspool", bufs=4, space="PSUM") as pspool,
        ):
            import bass_rust

            def after(deps, dma):
                # schedule `dma` only after `deps` complete: paces the load
                # stream so the critical first transfers get the full
                # aggregate DMA bandwidth instead of round-robin slices
                for dep in deps:
                    bass_rust.add_dep_helper(dma.ins, dep.ins, reason="dma pacing")

            prev_gate = None
            for ex in range(BP):
                # Loads go on SP, stores on ACT, seam placement on SWDGE
                # (DMA issue is serialized per DGE sequencer, ~0.7us each)
                wband = wpool.tile([P, WB_F], f32)
                wb_dma = nc.sync.dma_start(out=wband, in_=wband_d[ex])
                h_sb = hpool.tile([P, NT * H], f32)
                h_dmas = []
                for q in range(2):
                    c0, c1 = 4 * q * H, 4 * (q + 1) * H
                    h_dmas.append(
                        nc.sync.dma_start(out=h_sb[:, c0:c1], in_=h_d[ex, :, c0:c1])
                    )
                seam_rhs = srhs_pool.tile([SK, H], f32)
                hs_dma = nc.sync.dma_start(out=seam_rhs, in_=hs_d[ex])

                crit = [wb_dma, h_dmas[0]]
                after(crit, h_dmas[1])
                after(crit, hs_dma)
                if prev_gate is not None:
                    for dma in (wb_dma, h_dmas[0], h_dmas[1], hs_dma):
                        after([prev_gate], dma)
                prev_gate = h_dmas[1]
                psum_seam_full = pspool.tile([P, H], f32, tag="ps")
                psum_seam = psum_seam_full[0:SM, :]
                for c0, c1 in ((0, 512), (512, H)):
                    nc.tensor.matmul(
                        out=psum_seam[:, c0:c1],
                        lhsT=wband[0:SK, SEAM_OFF : SEAM_OFF + SM],
                        rhs=seam_rhs[:, c0:c1],
                        start=True,
                        stop=True,
                    )
                seam_sb = ssb_pool.tile([SM, H], f32)
                nc.scalar.copy(out=seam_sb, in_=psum_seam[:])

                out_sb = opool.tile([P, NT * 2 * H], f32)
                # place the boundary rows into the staged output (SWDGE;
                # engine partition-slices must be 32-aligned, DMA is free)
                for t in range(NT):
                    nc.gpsimd.dma_start(
                        out=out_sb[MT:P, t * 2 * H : t * 2 * H + H],
                        in_=seam_sb[SW * t : SW * (t + 1), :],
                    )

                for t in range(NT):
                    rhs = h_sb[:, t * H : (t + 1) * H]
                    lhsT = wband[:, t * MT : (t + 1) * MT]
                    ob = t * 2 * H
                    psum = pspool.tile([P, H], f32, tag="ps")
                    # fp32 matmul: moving operand <= 512 cols (one bank)
                    for c0, c1 in ((0, 512), (512, H)):
                        nc.tensor.matmul(
                            out=psum[0:MT, c0:c1],
                            lhsT=lhsT,
                            rhs=rhs[:, c0:c1],
                            start=True,
                            stop=True,
                        )
                    # alternate evacuation engine per tile to halve the
                    # per-tile psum-release turnaround
                    if t % 2 == 0:
                        nc.scalar.copy(
                            out=out_sb[0:MT, ob : ob + H], in_=psum[0:MT, :]
                        )
                    else:
                        nc.vector.tensor_copy(
                            out=out_sb[0:MT, ob : ob + H], in_=psum[0:MT, :]
                        )
                    wsum = spool.tile([P, 1], f32)
                    nc.vector.reduce_sum(
                        out=wsum,
                        in_=wband[:, BAND_OFF + t * D : BAND_OFF + (t + 1) * D],
                        axis=mybir.AxisListType.X,
                    )
                    nc.vector.tensor_scalar_mul(
                        out=out_sb[:, ob + H : ob + 2 * H],
                        in0=rhs,
                        scalar1=wsum,
                    )
                    if t % 2 == 1:
                        # store two blocks: 12KB per-partition descriptors
                        c0, c1 = (t - 1) * 2 * H, (t + 1) * 2 * H
                        nc.scalar.dma_start(
                            out=out_d[ex, :, c0:c1], in_=out_sb[:, c0:c1]
                        )
    nc.compile()
    return nc


def _host_prep(span_adjacency, bound_hidden):
    """Extract the used 6-wide diagonal band, build the banded matmul
    weights, and pack h partition-major. Pure gather/layout — no
    arithmetic on the data."""
    adj = span_adjacency.reshape(B, L, L)
    band = np.zeros((B, L, D), dtype=np.float32)
    for d in range(D):
        # band[b, j, d] = adj[b, j, j+d] for j+d < L, else 0
        band[:, : L - d, d] = np.diagonal(adj, offset=d, axis1=1, axis2=2)
    band_t = band.reshape(B, NT, P, D)

    # main lhsT[b, t, k, m] = band[b, 128t+m, k-m] for m < 123 (full windows)
    wa = np.zeros((B, NT, P, MT), dtype=np.float32)
    mm = np.arange(MT)
    for d in range(D):
        wa[:, :, mm + d, mm] = band_t[:, :, :MT, d]

    # seam lhsT[b, 10s+u, 5s+q] = band[b, 128s+123+q, u-q] for 0 <= u-q <= 5
    # (k row 10s+u is h row 128s+123+u; out row m=5s+q is j=128s+123+q)
    seam = np.zeros((B, SK, SM), dtype=np.float32)
    s = np.arange(NT)
    for q in range(SW):
        for u in range(q, q + D):
            seam[:, SR * s + u, SW * s + q] = band_t[:, s, MT + q, u - q]

    wband = np.zeros((B, P, WB_F), dtype=np.float32)
    wband[:, :, :WA_COLS] = wa.transpose(0, 2, 1, 3).reshape(B, P, NT * MT)
    wband[:, :, BAND_OFF:SEAM_OFF] = band_t.transpose(0, 2, 1, 3).reshape(B, P, NT * D)
    wband[:, :SK, SEAM_OFF:] = seam

    h32 = np.ascontiguousarray(bound_hidden, dtype=np.float32)
    # packed h: partition p, block t holds row 128t+p
    hpack = h32.reshape(B, NT, P, H).transpose(0, 2, 1, 3).reshape(B, P, NT * H)
    # seam h rows: 10 consecutive rows 128s+123 .. 128s+132 per boundary
    # (rows >= L are only multiplied by zero weights; use zeros)
    h_pad = np.zeros((B, NT * P + SR, H), dtype=np.float32)
    h_pad[:, :L] = h32
    idx = (P * np.arange(NT)[:, None] + MT + np.arange(SR)[None, :]).ravel()
    hseam = h_pad[:, idx, :]

    return [
        {
            "hpack": np.ascontiguousarray(hpack[BP * c : BP * (c + 1)]),
            "hseam": np.ascontiguousarray(hseam[BP * c : BP * (c + 1)]),
            "wband": np.ascontiguousarray(wband[BP * c : BP * (c + 1)]),
        }
        for c in range(NCORES)
    ]


def _host_unpack(outpacks):
    """outpack [BP, 128, NT*1536] per core -> out [B, L, 1536]."""
    op = np.concatenate(outpacks, axis=0)
    return np.ascontiguousarray(
        op.reshape(B, P, NT, 2 * H).transpose(0, 2, 1, 3).reshape(B, L, 2 * H)
    )


def run(span_adjacency, bound_hidden, trace=False):
    """Run on 8 NeuronCores; returns (out [B, L, 2H] f32, exec_time_ns|None)."""
    global _nc_cache
    from concourse import bass_utils

    in_maps = _host_prep(np.asarray(span_adjacency), np.asarray(bound_hidden))
    if _nc_cache is None:
        _nc_cache = _build_bass()
    res = bass_utils.run_bass_kernel_spmd(
        _nc_cache, in_maps, core_ids=list(range(NCORES)), trace=trace
    )
    out = _host_unpack([r["outpack"] for r in res.results])
    return out, res.exec_time_ns


def kernel(span_adjacency, bound_hidden):
    out, _ = run(span_adjacency, bound_hidden, trace=False)
    return out


# revision 25
# speedup vs baseline: 1.1822x; 1.1822x over previous
"""Trainium2 Bass kernel for nn_BoundarySeg (gnn_message_passing).

Computation (per example b, position j, MAX_SPAN_LEN=6 window):
    first[j]  = sum_{d=0..5, j+d<L} w[j, j+d] * h[j+d]
    second[j] = h[j] * sum_{d, j+d<L} w[j, j+d]
    out[j]    = concat([first, second])            # [B, L, 2H]

Only the 6-diagonal band of the [B, L, L] adjacency is ever used, so the
host extracts that band (a pure strided gather / data-layout step) and
builds small banded weight matrices; all arithmetic (the windowed weighted
sums and the scaled copy) runs on-device.

Device strategy (pure data parallel, B=16 sharded 2-per-core over 8 cores):
  - 128-aligned h blocks; tile t computes out rows [128t, 128t+123) as one
    banded matmul (lhsT[k, m] = band[128t+m, k-m], zero off the diagonals)
    against h block t. The remaining 5 boundary rows per block (whose
    windows straddle the block edge) are computed by ONE batched
    block-diagonal matmul per example over the 10 consecutive h rows at
    each boundary (K=80, M=40), then placed into the staged output tiles
    with small SBUF->SBUF DMAs (engine partition-slices must be 32-aligned
    on TRN2; DMA has no such restriction).
  - `second` as a per-partition tensor_scalar multiply on the Vector
    engine, with the window sums reduced on-device from the band.
  - DMA efficiency: h and out use partition-major packed DRAM layouts
    (host packs/unpacks) giving 24KB / 12KB contiguous descriptor runs,
    and every DMA's partition count is a multiple of 16 so the descriptor
    balancer spreads each transfer across all 16 SDMA engines.
  - HBM traffic per core ~20 MB (in ~7.5 MB + out 12.6 MB): memory-bound.
"""

import os
import sys

import numpy as np

if "/opt/trn_rl_repo" not in sys.path:
    sys.path.insert(0, "/opt/trn_rl_repo")

B, L, H = 16, 1024, 768
D = 6             # MAX_SPAN_LEN
NCORES = 8
BP = B // NCORES  # examples per core
P = 128
NT = L // P       # 8 aligned tiles per example
MT = P - (D - 1)  # 123 main-matmul output rows per tile
SW = D - 1        # 5 boundary rows per block
SR = 2 * SW       # 10 h rows feeding each boundary group
SK = NT * SR      # 80 rows in the batched seam matmul (K)
SM = NT * SW      # 40 seam output rows (M)

# wband column layout
WA_COLS = NT * MT           # 984
BAND_OFF = WA_COLS          # 8*6 = 48 band cols
SEAM_OFF = BAND_OFF + NT * D
WB_F = SEAM_OFF + SM        # 1072

OUT_PAIR = 2                # output tiles per store DMA (12KB descriptors)

_nc_cache = None


def _build_bass():
    import concourse.tile as tile
    from concourse import bacc, mybir

    f32 = mybir.dt.float32
    nc = bacc.Bacc("TRN2", target_bir_lowering=False)

    h_d = nc.dram_tensor("hpack", [BP, P, NT * H], f32, kind="ExternalInput")
    hs_d = nc.dram_tensor("hseam", [BP, SK, H], f32, kind="ExternalInput")
    wband_d = nc.dram_tensor("wband", [BP, P, WB_F], f32, kind="ExternalInput")
    out_d = nc.dram_tensor("outpack", [BP, P, NT * 2 * H], f32, kind="ExternalOutput")

    with tile.TileContext(nc) as tc:
        with (
            tc.tile_pool(name="hpool", bufs=2) as hpool,
            tc.tile_pool(name="wpool", bufs=2) as wpool,
            tc.tile_pool(name="srhs", bufs=2) as srhs_pool,
            tc.tile_pool(name="ssb", bufs=2) as ssb_pool,
            tc.tile_pool(name="opool", bufs=2) as opool,
            tc.tile_pool(name="spool", bufs=4) as spool,
            tc.tile_pool(name="pspool", bufs=4, space="PSUM") as pspool,
        ):
            import bass_rust

            for ex in range(BP):
                # Loads go on SP, stores on ACT, seam placement on SWDGE
                # (DMA issue is serialized per DGE sequencer, ~0.7us each)
                wband = wpool.tile([P, WB_F], f32)
                nc.sync.dma_start(out=wband, in_=wband_d[ex])
                h_sb = hpool.tile([P, NT * H], f32)
                for q in range(2):
                    c0, c1 = 4 * q * H, 4 * (q + 1) * H
                    nc.sync.dma_start(out=h_sb[:, c0:c1], in_=h_d[ex, :, c0:c1])
                seam_rhs = srhs_pool.tile([SK, H], f32)
                nc.sync.dma_start(out=seam_rhs, in_=hs_d[ex])

                out_sb = opool.tile([P, NT * 2 * H], f32)

                for t in range(NT):
                    if t == 2:
                        # seam matmul sits here in the PE FIFO: late enough
                        # that tiles 0-1 don't wait on the hseam load, early
                        # enough for the first paired store
                        psum_seam_full = pspool.tile([P, H], f32, tag="ps")
                        psum_seam = psum_seam_full[0:SM, :]
                        for c0, c1 in ((0, 512), (512, H)):
                            nc.tensor.matmul(
                                out=psum_seam[:, c0:c1],
                                lhsT=wband[0:SK, SEAM_OFF : SEAM_OFF + SM],
                                rhs=seam_rhs[:, c0:c1],
                                start=True,
                                stop=True,
                            )
                        seam_sb = ssb_pool.tile([SM, H], f32)
                        nc.scalar.copy(out=seam_sb, in_=psum_seam[:])
                        # place the boundary rows into the staged output
                        # (SWDGE; engine partition-slices must be 32-aligned
                        # on TRN2, DMA has no such restriction)
                        for ts in range(NT):
                            nc.gpsimd.dma_start(
                                out=out_sb[MT:P, ts * 2 * H : ts * 2 * H + H],
                                in_=seam_sb[SW * ts : SW * (ts + 1), :],
                            )
                    rhs = h_sb[:, t * H : (t + 1) * H]
                    lhsT = wband[:, t * MT : (t + 1) * MT]
                    ob = t * 2 * H
                    psum = pspool.tile([P, H], f32, tag="ps")
                    # fp32 matmul: moving operand <= 512 cols (one bank)
                    for c0, c1 in ((0, 512), (512, H)):
                        nc.tensor.matmul(
                            out=psum[0:MT, c0:c1],
                            lhsT=lhsT,
                            rhs=rhs[:, c0:c1],
                            start=True,
                            stop=True,
                        )
                    # alternate evacuation engine per tile to halve the
                    # per-tile psum-release turnaround
                    if t % 2 == 0:
                        nc.scalar.copy(
                            out=out_sb[0:MT, ob : ob + H], in_=psum[0:MT, :]
                        )
                    else:
                        nc.vector.tensor_copy(
                            out=out_sb[0:MT, ob : ob + H], in_=psum[0:MT, :]
                        )
                    wsum = spool.tile([P, 1], f32)
                    nc.vector.reduce_sum(
                        out=wsum,
                        in_=wband[:, BAND_OFF + t * D : BAND_OFF + (t + 1) * D],
                        axis=mybir.AxisListType.X,
                    )
                    nc.vector.tensor_scalar_mul(
                        out=out_sb[:, ob + H : ob + 2 * H],
                        in0=rhs,
                        scalar1=wsum,
                    )
                    # paired stores (12KB per-partition descriptors); pair 0
                    # is deferred to t=2 so it is recorded after the seam
                    # placement writes it must wait for
                    if t == 2 or (t % 2 == 1 and t >= 3):
                        c0 = (t - 1) * 2 * H if t % 2 == 1 else 0
                        c1 = c0 + 2 * (2 * H)
                        nc.scalar.dma_start(
                            out=out_d[ex, :, c0:c1], in_=out_sb[:, c0:c1]
                        )
    nc.compile()
    return nc


def _host_prep(span_adjacency, bound_hidden):
    """Extract the used 6-wide diagonal band, build the banded matmul
    weights, and pack h partition-major. Pure gather/layout — no
    arithmetic on the data."""
    adj = span_adjacency.reshape(B, L, L)
    band = np.zeros((B, L, D), dtype=np.float32)
    for d in range(D):
        # band[b, j, d] = adj[b, j, j+d] for j+d < L, else 0
        band[:, : L - d, d] = np.diagonal(adj, offset=d, axis1=1, axis2=2)
    band_t = band.reshape(B, NT, P, D)

    # main lhsT[b, t, k, m] = band[b, 128t+m, k-m] for m < 123 (full windows)
    wa = np.zeros((B, NT, P, MT), dtype=np.float32)
    mm = np.arange(MT)
    for d in range(D):
        wa[:, :, mm + d, mm] = band_t[:, :, :MT, d]

    # seam lhsT[b, 10s+u, 5s+q] = band[b, 128s+123+q, u-q] for 0 <= u-q <= 5
    # (k row 10s+u is h row 128s+123+u; out row m=5s+q is j=128s+123+q)
    seam = np.zeros((B, SK, SM), dtype=np.float32)
    s = np.arange(NT)
    for q in range(SW):
        for u in range(q, q + D):
            seam[:, SR * s + u, SW * s + q] = band_t[:, s, MT + q, u - q]

    wband = np.zeros((B, P, WB_F), dtype=np.float32)
    wband[:, :, :WA_COLS] = wa.transpose(0, 2, 1, 3).reshape(B, P, NT * MT)
    wband[:, :, BAND_OFF:SEAM_OFF] = band_t.transpose(0, 2, 1, 3).reshape(B, P, NT * D)
    wband[:, :SK, SEAM_OFF:] = seam

    h32 = np.ascontiguousarray(bound_hidden, dtype=np.float32)
    # packed h: partition p, block t holds row 128t+p
    hpack = h32.reshape(B, NT, P, H).transpose(0, 2, 1, 3).reshape(B, P, NT * H)
    # seam h rows: 10 consecutive rows 128s+123 .. 128s+132 per boundary
    # (rows >= L are only multiplied by zero weights; use zeros)
    h_pad = np.zeros((B, NT * P + SR, H), dtype=np.float32)
    h_pad[:, :L] = h32
    idx = (P * np.arange(NT)[:, None] + MT + np.arange(SR)[None, :]).ravel()
    hseam = h_pad[:, idx, :]

    return [
        {
            "hpack": np.ascontiguousarray(hpack[BP * c : BP * (c + 1)]),
            "hseam": np.ascontiguousarray(hseam[BP * c : BP * (c + 1)]),
            "wband": np.ascontiguousarray(wband[BP * c : BP * (c + 1)]),
        }
        for c in range(NCORES)
    ]


def _host_unpack(outpacks):
    """outpack [BP, 128, NT*1536] per core -> out [B, L, 1536]."""
    op = np.concatenate(outpacks, axis=0)
    return np.ascontiguousarray(
        op.reshape(B, P, NT, 2 * H).transpose(0, 2, 1, 3).reshape(B, L, 2 * H)
    )


def run(span_adjacency, bound_hidden, trace=False):
    """Run on 8 NeuronCores; returns (out [B, L, 2H] f32, exec_time_ns|None)."""
    global _nc_cache
    from concourse import bass_utils

    in_maps = _host_prep(np.asarray(span_adjacency), np.asarray(bound_hidden))
    if _nc_cache is None:
        _nc_cache = _build_bass()
    res = bass_utils.run_bass_kernel_spmd(
        _nc_cache, in_maps, core_ids=list(range(NCORES)), trace=trace
    )
    out = _host_unpack([r["outpack"] for r in res.results])
    return out, res.exec_time_ns


def kernel(span_adjacency, bound_hidden):
    out, _ = run(span_adjacency, bound_hidden, trace=False)
    return out


# revision 27
# speedup vs baseline: 1.2280x; 1.0388x over previous
"""Trainium2 Bass kernel for nn_BoundarySeg (gnn_message_passing).

Computation (per example b, position j, MAX_SPAN_LEN=6 window):
    first[j]  = sum_{d=0..5, j+d<L} w[j, j+d] * h[j+d]
    second[j] = h[j] * sum_{d, j+d<L} w[j, j+d]
    out[j]    = concat([first, second])            # [B, L, 2H]

Only the 6-diagonal band of the [B, L, L] adjacency is ever used, so the
host extracts that band (a pure strided gather / data-layout step) and
builds small banded weight matrices; all arithmetic (the windowed weighted
sums and the scaled copy) runs on-device.

Device strategy (pure data parallel, B=16 sharded 2-per-core over 8 cores):
  - 128-aligned h blocks; tile t computes out rows [128t, 128t+123) as one
    banded matmul (lhsT[k, m] = band[128t+m, k-m], zero off the diagonals)
    against h block t. The remaining 5 boundary rows per block (whose
    windows straddle the block edge) are computed by ONE batched
    block-diagonal matmul per example over the 10 consecutive h rows at
    each boundary (K=80, M=40), then placed into the staged output tiles
    with small SBUF->SBUF DMAs (engine partition-slices must be 32-aligned
    on TRN2; DMA has no such restriction).
  - `second` as a per-partition tensor_scalar multiply on the Vector
    engine, with the window sums reduced on-device from the band.
  - DMA efficiency: h and out use partition-major packed DRAM layouts
    (host packs/unpacks) giving 24KB / 12KB contiguous descriptor runs,
    and every DMA's partition count is a multiple of 16 so the descriptor
    balancer spreads each transfer across all 16 SDMA engines.
  - HBM traffic per core ~20 MB (in ~7.5 MB + out 12.6 MB): memory-bound.
"""

import os
import sys

import numpy as np

if "/opt/trn_rl_repo" not in sys.path:
    sys.path.insert(0, "/opt/trn_rl_repo")

B, L, H = 16, 1024, 768
D = 6             # MAX_SPAN_LEN
NCORES = 8
BP = B // NCORES  # examples per core
P = 128
NT = L // P       # 8 aligned tiles per example
MT = P - (D - 1)  # 123 main-matmul output rows per tile
SW = D - 1        # 5 boundary rows per block
SR = 2 * SW       # 10 h rows feeding each boundary group
SK = NT * SR      # 80 rows in the batched seam matmul (K)
SM = NT * SW      # 40 seam output rows (M)

# wband column layout
WA_COLS = NT * MT           # 984
BAND_OFF = WA_COLS          # 8*6 = 48 band cols
SEAM_OFF = BAND_OFF + NT * D
WB_F = SEAM_OFF + SM        # 1072

OUT_PAIR = 2                # output tiles per store DMA (12KB descriptors)

_nc_cache = None


def _build_bass():
    import concourse.tile as tile
    from concourse import bacc, mybir

    f32 = mybir.dt.float32
    nc = bacc.Bacc("TRN2", target_bir_lowering=False)

    h_d = nc.dram_tensor("hpack", [BP, P, NT * H], f32, kind="ExternalInput")
    hs_d = nc.dram_tensor("hseam", [BP, SK, H], f32, kind="ExternalInput")
    wband_d = nc.dram_tensor("wband", [BP, P, WB_F], f32, kind="ExternalInput")
    out_d = nc.dram_tensor("outpack", [BP, P, NT * 2 * H], f32, kind="ExternalOutput")

    with tile.TileContext(nc) as tc:
        with (
            tc.tile_pool(name="hpool", bufs=2) as hpool,
            tc.tile_pool(name="wpool", bufs=2) as wpool,
            tc.tile_pool(name="srhs", bufs=2) as srhs_pool,
            tc.tile_pool(name="ssb", bufs=2) as ssb_pool,
            tc.tile_pool(name="opool", bufs=2) as opool,
            tc.tile_pool(name="spool", bufs=4) as spool,
            tc.tile_pool(name="pspool", bufs=4, space="PSUM") as pspool,
        ):
            import bass_rust

            for ex in range(BP):
                # Loads go on SP, stores on ACT, seam placement on SWDGE
                # (DMA issue is serialized per DGE sequencer, ~0.7us each)
                wband = wpool.tile([P, WB_F], f32)
                nc.sync.dma_start(
                    out=wband[:].bitcast(mybir.dt.float32r),
                    in_=wband_d[ex].bitcast(mybir.dt.float32r),
                )
                h_sb = hpool.tile([P, NT * H], f32)
                for q in range(2):
                    c0, c1 = 4 * q * H, 4 * (q + 1) * H
                    nc.sync.dma_start(
                        out=h_sb[:, c0:c1].bitcast(mybir.dt.float32r),
                        in_=h_d[ex, :, c0:c1].bitcast(mybir.dt.float32r),
                    )
                seam_rhs = srhs_pool.tile([SK, H], f32)
                nc.sync.dma_start(
                    out=seam_rhs[:].bitcast(mybir.dt.float32r),
                    in_=hs_d[ex].bitcast(mybir.dt.float32r),
                )

                out_sb = opool.tile([P, NT * 2 * H], f32)

                for t in range(NT):
                    if t == 2:
                        # seam matmul sits here in the PE FIFO: late enough
                        # that tiles 0-1 don't wait on the hseam load, early
                        # enough for the first paired store
                        psum_seam_full = pspool.tile([P, H], f32, tag="ps")
                        psum_seam = psum_seam_full[0:SM, :]
                        for c0, c1 in ((0, 512), (512, H)):
                            nc.tensor.matmul(
                                out=psum_seam[:, c0:c1],
                                lhsT=wband[0:SK, SEAM_OFF : SEAM_OFF + SM].bitcast(
                                    mybir.dt.float32r
                                ),
                                rhs=seam_rhs[:, c0:c1].bitcast(mybir.dt.float32r),
                                start=True,
                                stop=True,
                            )
                        seam_sb = ssb_pool.tile([SM, H], f32)
                        nc.scalar.copy(out=seam_sb, in_=psum_seam[:])
                        # place the boundary rows into the staged output
                        # (SWDGE; engine partition-slices must be 32-aligned
                        # on TRN2, DMA has no such restriction)
                        for ts in range(NT):
                            nc.gpsimd.dma_start(
                                out=out_sb[MT:P, ts * 2 * H : ts * 2 * H + H],
                                in_=seam_sb[SW * ts : SW * (ts + 1), :],
                            )
                    rhs = h_sb[:, t * H : (t + 1) * H]
                    lhsT = wband[:, t * MT : (t + 1) * MT]
                    ob = t * 2 * H
                    psum = pspool.tile([P, H], f32, tag="ps")
                    # fp32 matmul: moving operand <= 512 cols (one bank)
                    for c0, c1 in ((0, 512), (512, H)):
                        nc.tensor.matmul(
                            out=psum[0:MT, c0:c1],
                            lhsT=lhsT.bitcast(mybir.dt.float32r),
                            rhs=rhs[:, c0:c1].bitcast(mybir.dt.float32r),
                            start=True,
                            stop=True,
                        )
                    # alternate evacuation engine per tile to halve the
                    # per-tile psum-release turnaround
                    if t % 2 == 0:
                        nc.scalar.copy(
                            out=out_sb[0:MT, ob : ob + H], in_=psum[0:MT, :]
                        )
                    else:
                        nc.vector.tensor_copy(
                            out=out_sb[0:MT, ob : ob + H], in_=psum[0:MT, :]
                        )
                    wsum = spool.tile([P, 1], f32)
                    nc.vector.reduce_sum(
                        out=wsum,
                        in_=wband[:, BAND_OFF + t * D : BAND_OFF + (t + 1) * D],
                        axis=mybir.AxisListType.X,
                    )
                    nc.vector.tensor_scalar_mul(
                        out=out_sb[:, ob + H : ob + 2 * H],
                        in0=rhs,
                        scalar1=wsum,
                    )
                    # paired stores (12KB per-partition descriptors); pair 0
                    # is deferred to t=2 so it is recorded after the seam
                    # placement writes it must wait for
                    if t == 2 or (t % 2 == 1 and t >= 3):
                        c0 = (t - 1) * 2 * H if t % 2 == 1 else 0
                        c1 = c0 + 2 * (2 * H)
                        nc.scalar.dma_start(
                            out=out_d[ex, :, c0:c1], in_=out_sb[:, c0:c1]
                        )
    nc.compile()
    return nc


def _host_prep(span_adjacency, bound_hidden):
    """Extract the used 6-wide diagonal band, build the banded matmul
    weights, and pack h partition-major. Pure gather/layout — no
    arithmetic on the data."""
    adj = span_adjacency.reshape(B, L, L)
    band = np.zeros((B, L, D), dtype=np.float32)
    for d in range(D):
        # band[b, j, d] = adj[b, j, j+d] for j+d < L, else 0
        band[:, : L - d, d] = np.diagonal(adj, offset=d, axis1=1, axis2=2)
    band_t = band.reshape(B, NT, P, D)

    # main lhsT[b, t, k, m] = band[b, 128t+m, k-m] for m < 123 (full windows)
    wa = np.zeros((B, NT, P, MT), dtype=np.float32)
    mm = np.arange(MT)
    for d in range(D):
        wa[:, :, mm + d, mm] = band_t[:, :, :MT, d]

    # seam lhsT[b, 10s+u, 5s+q] = band[b, 128s+123+q, u-q] for 0 <= u-q <= 5
    # (k row 10s+u is h row 128s+123+u; out row m=5s+q is j=128s+123+q)
    seam = np.zeros((B, SK, SM), dtype=np.float32)
    s = np.arange(NT)
    for q in range(SW):
        for u in range(q, q + D):
            seam[:, SR * s + u, SW * s + q] = band_t[:, s, MT + q, u - q]

    wband = np.zeros((B, P, WB_F), dtype=np.float32)
    wband[:, :, :WA_COLS] = wa.transpose(0, 2, 1, 3).reshape(B, P, NT * MT)
    wband[:, :, BAND_OFF:SEAM_OFF] = band_t.transpose(0, 2, 1, 3).reshape(B, P, NT * D)
    wband[:, :SK, SEAM_OFF:] = seam

    h32 = np.ascontiguousarray(bound_hidden, dtype=np.float32)
    # packed h: partition p, block t holds row 128t+p
    hpack = h32.reshape(B, NT, P, H).transpose(0, 2, 1, 3).reshape(B, P, NT * H)
    # seam h rows: 10 consecutive rows 128s+123 .. 128s+132 per boundary
    # (rows >= L are only multiplied by zero weights; use zeros)
    h_pad = np.zeros((B, NT * P + SR, H), dtype=np.float32)
    h_pad[:, :L] = h32
    idx = (P * np.arange(NT)[:, None] + MT + np.arange(SR)[None, :]).ravel()
    hseam = h_pad[:, idx, :]

    return [
        {
            "hpack": np.ascontiguousarray(hpack[BP * c : BP * (c + 1)]),
            "hseam": np.ascontiguousarray(hseam[BP * c : BP * (c + 1)]),
            "wband": np.ascontiguousarray(wband[BP * c : BP * (c + 1)]),
        }
        for c in range(NCORES)
    ]


def _host_unpack(outpacks):
    """outpack [BP, 128, NT*1536] per core -> out [B, L, 1536]."""
    op = np.concatenate(outpacks, axis=0)
    return np.ascontiguousarray(
        op.reshape(B, P, NT, 2 * H).transpose(0, 2, 1, 3).reshape(B, L, 2 * H)
    )


def run(span_adjacency, bound_hidden, trace=False):
    """Run on 8 NeuronCores; returns (out [B, L, 2H] f32, exec_time_ns|None)."""
    global _nc_cache
    from concourse import bass_utils

    in_maps = _host_prep(np.asarray(span_adjacency), np.asarray(bound_hidden))
    if _nc_cache is None:
        _nc_cache = _build_bass()
    res = bass_utils.run_bass_kernel_spmd(
        _nc_cache, in_maps, core_ids=list(range(NCORES)), trace=trace
    )
    out = _host_unpack([r["outpack"] for r in res.results])
    return out, res.exec_time_ns


def kernel(span_adjacency, bound_hidden):
    out, _ = run(span_adjacency, bound_hidden, trace=False)
    return out
